# revision 9
# baseline (speedup 1.0000x reference)
"""Trainium2 Bass kernel for BatchMultiHeadGraphAttention.

Reference computation (per batch b, head h):
    h_prime = h @ w[h]                              # [n, f_out]
    S       = h_prime @ (a_src[h] + a_dst[h])       # [n, n]   (attn_src+attn_dst)
    attn    = softmax(leaky_relu(S, 0.2), axis=-1)  # [n, n]
    out     = attn @ h_prime + bias                 # [n, f_out]

Sharding: data-parallel over batch (bs=8 -> one batch per NeuronCore).
Each core writes its full [H, n, n] attn slice and [H, n, f_out] out slice.

Key kernel ideas:
  * exp(leaky_relu(x)) == max(exp(x), exp(0.2*x))  (exact; monotone pieces),
    so leaky+exp is two ACT exp passes (the 0.2 rides the free scale) plus a
    DVE tensor_tensor_reduce(max) that also emits the softmax row-sums.
  * attn is produced in natural [i, j] layout (softmax axis in the free dim,
    contiguous HBM rows); the second matmul attn @ h_prime needs j on
    partitions, so normalized fp16 attn tiles are transposed with the DMA
    xbar (2-byte path) and contracted against fp16 h_prime.
  * out is accumulated transposed ([f_out, n] psum), bias is added during
    psum eviction (per-partition bias AP), and small PE transposes restore
    the [n, f_out] HBM layout.
"""

import sys
import numpy as np

if "/opt/trn_rl_repo" not in sys.path:
    sys.path.insert(0, "/opt/trn_rl_repo")

import concourse.bacc as bacc
import concourse.bass as bass
import concourse.tile as tile
from concourse import masks, mybir

F32 = mybir.dt.float32
F16 = mybir.dt.float16
AF = mybir.ActivationFunctionType
OP = mybir.AluOpType


def build_bass(H=8, N=1024, FIN=64, FOUT=32, n_cores=8, leaky_mode="prelu"):
    NCH = N // 128          # 128-row chunks of the score matrix
    HG = min(4, H)          # heads per setup matmul group (M = HG*FOUT <= 128)

    nc = bacc.Bacc(
        "TRN2", target_bir_lowering=False, debug=False, num_devices=n_cores
    )

    hT_d = nc.dram_tensor("hT", [FIN, N], F32, kind="ExternalInput").ap()
    w_d = nc.dram_tensor("w", [H, FIN, FOUT], F32, kind="ExternalInput").ap()
    asrc_d = nc.dram_tensor("a_src", [H, FOUT, N], F32, kind="ExternalInput").ap()
    adst_d = nc.dram_tensor("a_dst", [H, FOUT, N], F32, kind="ExternalInput").ap()
    bias_d = nc.dram_tensor("bias", [FOUT], F32, kind="ExternalInput").ap()
    attn_d = nc.dram_tensor("attn", [H, N, N], F32, kind="ExternalOutput").ap()
    out_d = nc.dram_tensor("out", [H, N, FOUT], F32, kind="ExternalOutput").ap()

    with tile.TileContext(nc) as tc:
        with (
            tc.tile_pool(name="const", bufs=1) as cpool,
            tc.tile_pool(name="work", bufs=3) as wpool,
            tc.tile_pool(name="at", bufs=2) as atpool,
            tc.tile_pool(name="ps_s", bufs=2, space="PSUM") as ps_s,
            tc.tile_pool(name="ps_o", bufs=1, space="PSUM") as ps_o,
            tc.tile_pool(name="ps_t", bufs=2, space="PSUM") as ps_t,
        ):
            # ---------------- setup ----------------
            hT = cpool.tile([FIN, N], F32)                 # h[b]^T
            nc.sync.dma_start(out=hT, in_=hT_d)
            w_sb = cpool.tile([FIN, H * FOUT], F32)        # [f, h*FOUT+o]
            nc.sync.dma_start(
                out=w_sb.rearrange("f (h o) -> f h o", h=H),
                in_=w_d.rearrange("h f o -> f h o"),
            )
            a_comb = cpool.tile([FOUT, H * N], F32)        # a_src+a_dst, [o, h*N+j]
            a_tmp = cpool.tile([FOUT, H * N], F32)
            nc.sync.dma_start(
                out=a_comb.rearrange("p (h n) -> p h n", h=H),
                in_=asrc_d.rearrange("h p n -> p h n"),
            )
            nc.sync.dma_start(
                out=a_tmp.rearrange("p (h n) -> p h n", h=H),
                in_=adst_d.rearrange("h p n -> p h n"),
            )
            nc.vector.tensor_tensor(out=a_comb, in0=a_comb, in1=a_tmp, op=OP.add)
            bias_sb = cpool.tile([FOUT, 1], F32)
            nc.sync.dma_start(out=bias_sb, in_=bias_d)
            idn = cpool.tile([128, 128], F32)
            masks.make_identity(nc, idn)
            al02 = cpool.tile([128, 1], F32)
            nc.vector.memset(al02, 0.2)

            # h_primeT[h] = (h @ w[h])^T : [FOUT, N], packed [o, h*N+i]
            hpT = cpool.tile([FOUT, H * N], F32)
            for hg in range(H // HG):
                for j0 in range(0, N, 512):
                    jn = min(512, N - j0)
                    pss = ps_o.tile([HG * FOUT, jn], F32, tag="OT")
                    nc.tensor.matmul(
                        pss,
                        lhsT=w_sb[:, hg * HG * FOUT:(hg + 1) * HG * FOUT],
                        rhs=hT[:, j0:j0 + jn],
                    )
                    for hh in range(HG):
                        h_idx = hg * HG + hh
                        nc.scalar.copy(
                            out=hpT[:, h_idx * N + j0: h_idx * N + j0 + jn],
                            in_=pss[hh * FOUT:(hh + 1) * FOUT, :],
                        )

            # h_prime in fp16, [j(part), jc*(H*FOUT) + h*FOUT + o]
            hp16 = cpool.tile([128, NCH * H * FOUT], F16)
            for jc in range(NCH):
                pst = ps_t.tile([128, H * FOUT], F32, tag="tr")
                nc.tensor.matmul(pst, lhsT=hT[:, jc * 128:(jc + 1) * 128], rhs=w_sb)
                nc.vector.tensor_copy(
                    out=hp16[:, jc * H * FOUT:(jc + 1) * H * FOUT], in_=pst
                )

            # ---------------- main loop ----------------
            for h in range(H):
                AT = atpool.tile([128, NCH * N], F16, tag="AT")   # attn^T tiles
                rs = wpool.tile([128, NCH], F32, tag="rs")
                rr = wpool.tile([128, NCH], F32, tag="rr")
                for ic in range(NCH):
                    S = ps_s.tile([128, N], F32, tag="S")
                    for j0 in range(0, N, 512):
                        jn = min(512, N - j0)
                        nc.tensor.matmul(
                            S[:, j0:j0 + jn],
                            lhsT=hpT[:, h * N + ic * 128: h * N + (ic + 1) * 128],
                            rhs=a_comb[:, h * N + j0: h * N + j0 + jn],
                        )
                    E = wpool.tile([128, N], F32, tag="E")
                    if leaky_mode == "prelu":
                        # leaky_relu in one ACT pass (alpha honored only via
                        # the per-partition Prelu alpha AP on this HW), then
                        # exp with fused row-sum accumulation.
                        L = wpool.tile([128, N], F32, tag="L")
                        nc.scalar.activation(L, S, AF.Prelu, alpha=al02)
                        nc.scalar.activation(
                            E, L, AF.Exp, accum_out=rs[:, ic:ic + 1]
                        )
                    else:
                        # simulator-friendly fallback: exp(leaky(x)) ==
                        # max(exp(x), exp(0.2x)), rowsum via TTR.
                        e1 = wpool.tile([128, N], F32, tag="e1")
                        e2 = wpool.tile([128, N], F32, tag="e2")
                        nc.scalar.activation(e1, S, AF.Exp)
                        nc.scalar.activation(e2, S, AF.Exp, scale=0.2)
                        nc.vector.tensor_tensor_reduce(
                            out=E, in0=e1, in1=e2, scale=1.0, scalar=0.0,
                            op0=OP.max, op1=OP.add, accum_out=rs[:, ic:ic + 1],
                        )
                    nc.vector.reciprocal(rr[:, ic:ic + 1], rs[:, ic:ic + 1])
                    attnf = wpool.tile([128, N], F32, tag="attnf")
                    nc.vector.tensor_scalar(
                        out=attnf, in0=E, scalar1=rr[:, ic:ic + 1], scalar2=None,
                        op0=OP.mult,
                    )
                    nc.sync.dma_start(
                        out=attn_d[h, ic * 128:(ic + 1) * 128, :], in_=attnf
                    )
                    attn16 = wpool.tile([128, N], F16, tag="attn16")
                    nc.vector.tensor_scalar(
                        out=attn16, in0=E, scalar1=rr[:, ic:ic + 1], scalar2=None,
                        op0=OP.mult,
                    )
                    for jc in range(NCH):
                        nc.sync.dma_start(
                            out=AT[:, jc * N + ic * 128: jc * N + (ic + 1) * 128],
                            in_=attn16[:, jc * 128:(jc + 1) * 128],
                            transpose=True,
                        )

                # out^T = h_prime^T @ attn^T, accumulated over j chunks
                OT = ps_o.tile([FOUT, N], F32, tag="OT")
                for j0 in range(0, N, 512):
                    jn = min(512, N - j0)
                    for jc in range(NCH):
                        nc.tensor.matmul(
                            OT[:, j0:j0 + jn],
                            lhsT=hp16[:, jc * H * FOUT + h * FOUT:
                                      jc * H * FOUT + (h + 1) * FOUT],
                            rhs=AT[:, jc * N + j0: jc * N + j0 + jn],
                            start=(jc == 0), stop=(jc == NCH - 1),
                        )
                OTs = wpool.tile([FOUT, N], F32, tag="OTs")
                nc.scalar.activation(OTs, OT, AF.Identity, bias=bias_sb)

                # transpose out^T back to [i, o] in 128-row blocks
                pst = ps_t.tile([128, NCH * FOUT], F32, tag="tr")
                for ic in range(NCH):
                    nc.tensor.matmul(
                        pst[:, ic * FOUT:(ic + 1) * FOUT],
                        lhsT=OTs[:, ic * 128:(ic + 1) * 128],
                        rhs=idn[0:FOUT, 0:FOUT],
                        is_transpose=True,
                        skip_group_check=True,
                    )
                outb = wpool.tile([128, NCH * FOUT], F32, tag="outb")
                nc.vector.tensor_copy(out=outb, in_=pst)
                nc.sync.dma_start(
                    out=out_d[h].rearrange("(ic p) o -> p ic o", p=128),
                    in_=outb.rearrange("p (ic o) -> p ic o", o=FOUT),
                )
    nc.compile()
    return nc


_CACHE = {}


def _get_built():
    if "nc" not in _CACHE:
        _CACHE["nc"] = build_bass()
    return _CACHE["nc"]


def kernel(h, w, a_src, a_dst, bias):
    h = np.asarray(h, dtype=np.float32)
    w = np.asarray(w, dtype=np.float32)
    a_src = np.asarray(a_src, dtype=np.float32)
    a_dst = np.asarray(a_dst, dtype=np.float32)
    bias = np.asarray(bias, dtype=np.float32)
    bs = h.shape[0]

    from concourse import bass_utils

    nc = _get_built()
    in_maps = []
    for b in range(bs):
        in_maps.append({
            "hT": np.ascontiguousarray(h[b].T),
            "w": w,
            "a_src": a_src,
            "a_dst": a_dst,
            "bias": bias,
        })
    res = bass_utils.run_bass_kernel_spmd(nc, in_maps, core_ids=list(range(bs)))
    _CACHE["last_results"] = res
    out = np.stack([res.results[b]["out"] for b in range(bs)])
    attn = np.stack([res.results[b]["attn"] for b in range(bs)])
    return out, attn
